# revision 1
# baseline (speedup 1.0000x reference)
"""CFConv Trainium2 kernel.

Math: out[b,o,y,x] = sum_{k,i,j} weight[k,o,i,j] * fa[b,i,y+dy,x+dx] * wa[b,j,y+dy,x+dx]
(3x3 valid conv over the outer-product channel space of fa (65ch) x wa (17ch)).

Strategy (8 NeuronCores, SPMD):
- Shard (batch b, row-half h): each core computes 63 output rows of one batch.
- On-chip, form z[(i,j), pix] = f_i * w_j for the 64x16 "main" (i,j) grid as
  8 partition-chunks of 128 (j-major within 16-partition groups), built by a
  stream_shuffle (replicates f rows 16x using a per-quadrant lane mask) and an
  elementwise multiply against a pre-tiled copy of w. The remaining 81
  channels (j=16 ones column, i=64 ones row, corner) are read directly from a
  packed [f; w; ones] tensor.
- Contract with the tensor engine in fp16 (fp32 PSUM accumulation). Matmuls
  are issued in column-tiled pairs (tile_position (0,0)/(0,64)): the two
  64-wide PE column groups concurrently compute two adjacent 512-pixel output
  tiles, accumulating into the lower/upper partition halves of one PSUM bank.
- Output layout stays at input width (128) so all 9 conv offsets are plain
  column shifts; the two garbage columns per row are skipped at DMA-out.
"""

import numpy as np

B, WCH, FCH, OCH, H, W = 4, 16, 64, 64, 128, 128
KX = 3
HO = WO = H - KX + 1          # 126
ROWS_OUT = 63                 # output rows per core
ROWS_IN = 65                  # input rows per core
FREE = 8448                   # padded region width (66 rows * 128)
VALID = ROWS_IN * W           # 8320
OUT_COLS = ROWS_OUT * W       # 8064
NPAIR = 8                     # pixel-tile pairs per core
HALO = 2 * W + 2              # 258

_cache = {}


def _build_program():
    import concourse.bacc as bacc
    import concourse.mybir as mybir
    import concourse.tile as tile

    f16 = mybir.dt.float16
    f32 = mybir.dt.float32

    nc = bacc.Bacc("TRN2", target_bir_lowering=False)
    fw_d = nc.dram_tensor("fw", (81, FREE), f16, kind="ExternalInput")
    fpre_d = nc.dram_tensor("fpre", (128, FREE), f16, kind="ExternalInput")
    wt_d = nc.dram_tensor("wt", (128, FREE), f16, kind="ExternalInput")
    wkm_d = nc.dram_tensor("wkm", (128, 9 * 8 * 64), f16, kind="ExternalInput")
    wkx_d = nc.dram_tensor("wkx", (81, 9 * 64), f16, kind="ExternalInput")
    out_d = nc.dram_tensor("out", (OCH, ROWS_OUT, WO), f32, kind="ExternalOutput")

    with tile.TileContext(nc) as tc:
        with tc.tile_pool(name="inp", bufs=1) as inp, \
             tc.tile_pool(name="frep", bufs=4) as freps, \
             tc.tile_pool(name="z", bufs=3) as zp, \
             tc.tile_pool(name="st", bufs=3) as stp, \
             tc.tile_pool(name="ps", bufs=4, space="PSUM") as psp:
            # dummy matmuls warm the PE clock (HAM) while the input DMAs
            # land; fed by a tiny early DMA, their PSUM bank is never read.
            warm = inp.tile([128, 256], f16)
            nc.sync.dma_start(warm[:], fpre_d[:, 0:256])
            warm_ps = psp.tile([128, 512], f32)
            for _ in range(16):
                nc.tensor.matmul(warm_ps[0:64, 0:256], warm[:, 0:64], warm[:, 0:256],
                                 start=True, stop=True, tile_position=(0, 0))

            fw_s = inp.tile([81, FREE], f16)
            fpre_s = inp.tile([128, FREE], f16)
            wt_s = inp.tile([128, FREE], f16)
            wkm_s = inp.tile([128, 9 * 8 * 64], f16)
            wkx_s = inp.tile([81, 9 * 64], f16)

            # first-pair data first; spread DMA *issue* across SP + ACT queues
            first = 1344
            nc.sync.dma_start(fw_s[:, 0:first], fw_d[:, 0:first])
            nc.scalar.dma_start(wkx_s[:], wkx_d[:])
            nc.sync.dma_start(fpre_s[:, 0:first // 2], fpre_d[:, 0:first // 2])
            nc.scalar.dma_start(fpre_s[:, first // 2:first], fpre_d[:, first // 2:first])
            nc.sync.dma_start(wt_s[:, 0:first // 2], wt_d[:, 0:first // 2])
            nc.scalar.dma_start(wt_s[:, first // 2:first], wt_d[:, first // 2:first])
            nc.scalar.dma_start(wkm_s[:, 0:1152], wkm_d[:, 0:1152])
            nc.sync.dma_start(wkm_s[:, 1152:], wkm_d[:, 1152:])
            nchunk = 3
            cw = (FREE - first) // nchunk
            for ch in range(nchunk):
                sl = slice(first + ch * cw, first + (ch + 1) * cw if ch < nchunk - 1 else FREE)
                nc.sync.dma_start(fpre_s[:, sl], fpre_d[:, sl])
                nc.scalar.dma_start(wt_s[:, sl], wt_d[:, sl])
                nc.sync.dma_start(fw_s[:, sl], fw_d[:, sl])

            for a in range(NPAIR):
                # pair 7 overlaps pair 6 by one row (cols 7040..8063) so every
                # matmul is a full N=512; the duplicated row is not stored.
                c0 = 1024 * a if a < NPAIR - 1 else 7040
                n0 = 512
                n1 = 512
                win = n0 + n1 + HALO
                zs = []
                for c in range(8):
                    mask = [2 * c + (r // 16) for r in range(32)]
                    frep = freps.tile([128, win], f16, tag="frep")
                    nc.vector.stream_shuffle(frep[:], fpre_s[:, c0:c0 + win], mask)
                    z = zp.tile([128, win], f16, tag=f"z{c}")
                    nc.vector.tensor_mul(z[:], frep[:], wt_s[:, c0:c0 + win])
                    zs.append(z)

                ps = psp.tile([128, 512], f32)
                for c in (8, 0, 1, 2, 3, 4, 5, 6, 7):
                    for k in range(9):
                        dy, dx = divmod(k, KX)
                        d = dy * W + dx
                        for g, (n, off) in enumerate(((n0, 0), (n1, 512))):
                            if c < 8:
                                lhsT = wkm_s[:, (c * 9 + k) * 64:(c * 9 + k) * 64 + 64]
                                rhs = zs[c][:, d + off:d + off + n]
                            else:
                                lhsT = wkx_s[:, k * 64:k * 64 + 64]
                                rhs = fw_s[:, c0 + d + off:c0 + d + off + n]
                            nc.tensor.matmul(
                                ps[64 * g:64 * g + 64, 0:n], lhsT, rhs,
                                start=(c == 8 and k == 0),
                                stop=(c == 7 and k == 8),
                                tile_position=(0, 64 * g),
                            )

                stage = stp.tile([128, 512], f32)
                nc.vector.tensor_copy(stage[:], ps[:])
                for g in (0, 1):
                    if a < NPAIR - 1:
                        r_dst, col_lo, nrow = 8 * a + 4 * g, 0, 4
                    elif g == 0:
                        r_dst, col_lo, nrow = 56, 128, 3   # drop duplicated row 55
                    else:
                        r_dst, col_lo, nrow = 59, 0, 4
                    src = stage[64 * g:64 * g + 64, col_lo:col_lo + nrow * W].rearrange(
                        "p (r c) -> p r c", c=W)[:, :, 0:WO]
                    nc.scalar.dma_start(out_d[:, r_dst:r_dst + nrow, :], src)

    nc.finalize()
    return nc


def _prep_core(inputf, inputw, b, h):
    r0 = 63 * h
    f_reg = np.zeros((64, FREE), np.float16)
    f_reg[:, :VALID] = inputf[b, :, r0:r0 + ROWS_IN, :].reshape(64, VALID)
    w_reg = np.zeros((16, FREE), np.float16)
    w_reg[:, :VALID] = inputw[b, :, r0:r0 + ROWS_IN, :].reshape(16, VALID)
    ones_reg = np.zeros((1, FREE), np.float16)
    ones_reg[0, :VALID] = 1.0
    fw = np.concatenate([f_reg, w_reg, ones_reg], 0)

    fpre = np.zeros((128, FREE), np.float16)
    q = np.arange(4)[:, None]
    s = np.arange(16)[None, :]
    rows = (8 * (s // 2) + 2 * q + (s % 2)).reshape(-1)        # [64]
    idx = (32 * q + s).reshape(-1)                             # [64]
    fpre[idx] = f_reg[rows]

    wt = np.empty((128, FREE), np.float16)
    for u in range(8):
        wt[16 * u:16 * u + 16] = w_reg
    return fw, fpre, wt


def kernel(inputw, inputf, weight):
    from concourse import bass_utils

    inputw = np.asarray(inputw, np.float32)
    inputf = np.asarray(inputf, np.float32)
    weight = np.asarray(weight, np.float32)

    if "nc" not in _cache:
        _cache["nc"] = _build_program()
    nc = _cache["nc"]

    # weight layouts (replicated across cores)
    p = np.arange(128)
    wkm = np.empty((128, 8, 9, 64), np.float16)
    for t in range(8):
        iw = 8 * t + p // 16
        jw = p % 16
        wkm[:, t, :, :] = weight[:, :, iw, jw].transpose(2, 0, 1)
    wkm = wkm.reshape(128, 8 * 9 * 64)
    wkx = np.empty((81, 9, 64), np.float16)
    wkx[:64] = weight[:, :, :64, 16].transpose(2, 0, 1)
    wkx[64:80] = weight[:, :, 64, :16].transpose(2, 0, 1)
    wkx[80] = weight[:, :, 64, 16]
    wkx = wkx.reshape(81, 9 * 64)

    in_maps = []
    for core in range(8):
        b, h = divmod(core, 2)
        fw, fpre, wt = _prep_core(inputf, inputw, b, h)
        in_maps.append({"fw": fw, "fpre": fpre, "wt": wt, "wkm": wkm, "wkx": wkx})

    res = bass_utils.run_bass_kernel_spmd(nc, in_maps, core_ids=list(range(8)))
    kernel.last_result = res

    out = np.empty((B, OCH, HO, WO), np.float32)
    for core in range(8):
        b, h = divmod(core, 2)
        out[b, :, 63 * h:63 * h + 63, :] = res.results[core]["out"]
    return out

